# revision 6
# baseline (speedup 1.0000x reference)
"""Distributed Trainium2 Bass kernel for a causal attention block + LayerNorm.

Reference computation (B=2, T=2048, C=1024, H=16 heads, Dh=64):
    q,k,v = x@Wq+bq, x@Wk+bk, x@Wv+bv          (per-head split)
    att   = softmax(causal(q k^T / sqrt(Dh)))
    o     = att @ v ; y = o@Wo + bo ; out = LayerNorm(y) * gamma + beta

Sharding (8 cores, one TRN2 chip):
    Tensor-parallel over heads: core i owns heads {2i, 2i+1} for BOTH
    batches (Megatron-style column shards of Wq/Wk/Wv).  After attention,
    one 8-core AllToAll redistributes the per-head outputs (plus softmax
    denominators) to token-sharding: core i ends with tokens
    [b = i//4, t in (i%4)*512 ...] with ALL 1024 features, then applies the
    softmax division, output projection (full Wo), bias and LayerNorm
    locally, and writes its (512, 1024) slice of the output.

Layout choices (all on-chip matmuls contract over the partition axis):
    - activations are feature-major: host passes x^T [C, B, T].
    - q^T,k^T [d, t] produced directly; v produced s-major [s, d] with an
      extra ones column per head ("v-hat") so the P@V matmul also yields the
      softmax denominator for free (row 64 of each 65-row block).
    - scores are computed transposed: S^T[s, q] = k^T.T @ q^T, exp on the
      scalar engine straight out of PSUM (scale=1/8 folded in), causal
      masking via gpsimd.affine_select on the 128x128 diagonal tiles only,
      and P@V uses v-hat as the stationary operand so the unnormalized
      attention output O^T [d, q] is produced feature-major (no transposes
      anywhere in the kernel).
"""

import os
import numpy as np
import ml_dtypes

import concourse.bass as bass
import concourse.mybir as mybir
import concourse.tile as tile
from concourse import bacc
from concourse.bass_utils import run_bass_kernel_spmd

F32 = mybir.dt.float32
BF16 = mybir.dt.bfloat16
AF = mybir.ActivationFunctionType
OP = mybir.AluOpType

B, T, C, H, Dh = 2, 2048, 1024, 16, 64
NCORES = 8
HPC = 2               # heads per core
DPC = HPC * Dh        # 128 feature columns per core
TS = 512              # output token-slice length per core
NQB = T // 512        # 4 q blocks
NST = T // 128        # 16 s tiles
NCT = C // 128        # 8 contraction tiles
EPS = 1e-5

# compute dtypes (numpy + bir); fp32 softmax/psums throughout
DT_X = BF16
DT_W = BF16
DT_P = BF16
NP_X = ml_dtypes.bfloat16
NP_W = ml_dtypes.bfloat16

_CACHE = {}


def _build():
    nc = bacc.Bacc("TRN2", target_bir_lowering=False, debug=False,
                   num_devices=NCORES)

    xT_h = nc.dram_tensor("xT", [C, B, T], DT_X, kind="ExternalInput")
    wq_h = nc.dram_tensor("wq", [C, DPC], DT_W, kind="ExternalInput")
    wk_h = nc.dram_tensor("wk", [C, DPC], DT_W, kind="ExternalInput")
    wv_h = nc.dram_tensor("wv", [C, DPC], DT_W, kind="ExternalInput")
    wo_h = nc.dram_tensor("wo", [C, C], DT_W, kind="ExternalInput")
    bqT_h = nc.dram_tensor("bqT", [DPC, 1], F32, kind="ExternalInput")
    bkT_h = nc.dram_tensor("bkT", [DPC, 1], F32, kind="ExternalInput")
    bvb_h = nc.dram_tensor("bvb", [128, HPC, 65], F32, kind="ExternalInput")
    bo_h = nc.dram_tensor("bo_row", [1, C], F32, kind="ExternalInput")
    gam_h = nc.dram_tensor("gamb", [128, C], F32, kind="ExternalInput")
    bet_h = nc.dram_tensor("betb", [128, C], F32, kind="ExternalInput")
    out_h = nc.dram_tensor("out", [TS, C], F32, kind="ExternalOutput")

    sel2_np = np.zeros((2, 128), np.float32)
    sel2_np[0, :64] = 1.0
    sel2_np[1, 64:] = 1.0
    sel2_d = nc.inline_tensor(sel2_np, name="sel2_const")
    ones1_d = nc.inline_tensor(np.ones((1, 128), np.float32), name="ones1_const")

    with tile.TileContext(nc) as tc:
        with (
            tc.tile_pool(name="const", bufs=1) as cp,
            tc.tile_pool(name="dram", bufs=1, space="DRAM") as dp,
            tc.tile_pool(name="xw", bufs=1) as xw,
            tc.tile_pool(name="act", bufs=1) as ap,
            tc.tile_pool(name="pp", bufs=6) as pp,
            tc.tile_pool(name="ohp", bufs=3) as ohp,
            tc.tile_pool(name="orp", bufs=3) as orp,
            tc.tile_pool(name="lnp", bufs=2) as lnp,
            tc.tile_pool(name="psP", bufs=4, space="PSUM") as psP,
            tc.tile_pool(name="psV", bufs=2, space="PSUM") as psV,
            tc.tile_pool(name="psO", bufs=2, space="PSUM") as psO,
        ):
            # ---- constants ----
            sel2 = cp.tile([2, 128], F32)
            nc.sync.dma_start(sel2[:], sel2_d[:])
            ones1 = cp.tile([1, 128], F32)
            nc.sync.dma_start(ones1[:], ones1_d[:])
            bqT = cp.tile([DPC, 1], F32)
            nc.sync.dma_start(bqT[:], bqT_h[:])
            bkT = cp.tile([DPC, 1], F32)
            nc.sync.dma_start(bkT[:], bkT_h[:])
            bvb = cp.tile([128, HPC, 65], F32)
            nc.sync.dma_start(bvb[:], bvb_h[:])
            bo = cp.tile([1, C], F32)
            nc.sync.dma_start(bo[:], bo_h[:])
            gam = cp.tile([128, C], F32)
            nc.sync.dma_start(gam[:], gam_h[:])
            bet = cp.tile([128, C], F32)
            nc.sync.dma_start(bet[:], bet_h[:])
            eps_t = cp.tile([128, 1], F32)
            nc.gpsimd.memset(eps_t[:], EPS)

            a2a_in = dp.tile([NCORES, HPC * 65, 512], F32)
            a2a_out = dp.tile([NCORES, HPC * 65, 512], F32)

            # ---- load x^T and weights ----
            xT = xw.tile([128, NCT, B, T], DT_X)
            xsrc = xT_h[:].rearrange("(ct p) b t -> p ct b t", p=128)
            for ct in range(NCT):
                nc.sync.dma_start(xT[:, ct], xsrc[:, ct])
            wq = xw.tile([128, NCT, DPC], DT_W)
            wk = xw.tile([128, NCT, DPC], DT_W)
            wv = xw.tile([128, NCT, DPC], DT_W)
            for w_sb, w_h in ((wq, wq_h), (wk, wk_h), (wv, wv_h)):
                nc.sync.dma_start(
                    w_sb[:], w_h[:].rearrange("(ct p) m -> p ct m", p=128))
            wo = xw.tile([128, NCT, C], DT_W)
            nc.sync.dma_start(
                wo[:], wo_h[:].rearrange("(ct p) m -> p ct m", p=128))

            # ---- phase 1: projections (feature-major q^T,k^T; s-major v) --
            qT = ap.tile([DPC, B, T], DT_P)
            kT = ap.tile([DPC, B, T], DT_P)
            vhat = ap.tile([128, B, NST, HPC, 65], DT_P)
            for b in range(B):
                nc.gpsimd.memset(vhat[:, b, :, :, 64:65], 1.0)

            for b in range(B):
                for qb in range(NQB):
                    for w_sb, bias, dst in ((wq, bqT, qT), (wk, bkT, kT)):
                        ps = psP.tile([128, 512], F32, tag="m")
                        for ct in range(NCT):
                            nc.tensor.matmul(
                                ps[:], w_sb[:, ct], xT[:, ct, b, qb * 512:(qb + 1) * 512],
                                start=(ct == 0), stop=(ct == NCT - 1))
                        nc.vector.tensor_scalar_add(
                            dst[:, b, qb * 512:(qb + 1) * 512], ps[:], bias[:])
                for st in range(NST):
                    ps = psV.tile([128, DPC], F32, tag="v")
                    for ct in range(NCT):
                        nc.tensor.matmul(
                            ps[:], xT[:, ct, b, st * 128:(st + 1) * 128], wv[:, ct],
                            start=(ct == 0), stop=(ct == NCT - 1))
                    nc.vector.tensor_tensor(
                        vhat[:, b, st, :, 0:64],
                        ps[:].rearrange("p (hh d) -> p hh d", hh=HPC),
                        bvb[:, :, 0:64], op=OP.add)

            # ---- phase 2: attention, S^T layout ----
            for b in range(B):
                for hh in range(HPC):
                    hlo = hh * 64
                    for qb in range(NQB):
                        o_ps = psO.tile([65, 512], F32, tag="o")
                        nsi = 4 * qb + 4
                        for si in range(nsi):
                            off = si * 128 - qb * 512
                            lo = max(off, 0)
                            w = 512 - lo
                            s_ps = psP.tile([128, w], F32, tag="m")
                            nc.tensor.matmul(
                                s_ps[:],
                                kT[hlo:hlo + 64, b, si * 128:(si + 1) * 128],
                                qT[hlo:hlo + 64, b, qb * 512 + lo:(qb + 1) * 512],
                                start=True, stop=True)
                            p_sb = pp.tile([128, w], DT_P, tag="p")
                            nc.scalar.activation(p_sb[:], s_ps[:], AF.Exp, scale=0.125)
                            if off >= 0:
                                # causal triangle on the 128-wide diagonal block
                                nc.gpsimd.affine_select(
                                    out=p_sb[:, 0:128], in_=p_sb[:, 0:128],
                                    compare_op=OP.is_ge, fill=0.0, base=0,
                                    channel_multiplier=-1, pattern=[[1, 128]])
                            nc.tensor.matmul(
                                o_ps[:, lo:512], vhat[:, b, si, hh, :], p_sb[:],
                                start=(si == 0), stop=(si == nsi - 1))
                        oh = ohp.tile([65, 512], F32, tag="oh")
                        nc.vector.tensor_copy(oh[:], o_ps[:])
                        nc.sync.dma_start(
                            a2a_in[b * 4 + qb, hh * 65:(hh + 1) * 65, :], oh[:])

            # ---- phase 3: AllToAll -> token sharding ----
            nc.gpsimd.collective_compute(
                "AllToAll", OP.bypass,
                replica_groups=[list(range(NCORES))],
                ins=[a2a_in.opt()], outs=[a2a_out.opt()])

            # ---- phase 4: softmax division, out-proj, LayerNorm ----
            # denom[j, ft, n] = softmax denominator of head 2*ft+j at token n
            denom = cp.tile([2, NCORES, 512], F32)
            nc.sync.dma_start(
                denom[:],
                a2a_out[:].rearrange("a (hh r) n -> hh a r n", r=65)[:, :, 64, :])
            rd = cp.tile([2, NCORES, 512], F32)
            nc.vector.reciprocal(rd[:], denom[:])

            oT = ap.tile([128, NCT, 512], DT_P)
            for ft in range(NCT):
                o_raw = orp.tile([128, 512], F32, tag="oraw")
                for hhh in range(2):
                    nc.sync.dma_start(
                        o_raw[hhh * 64:(hhh + 1) * 64, :],
                        a2a_out[ft, hhh * 65:hhh * 65 + 64, :])
                bc = psP.tile([128, 512], F32, tag="m")
                nc.tensor.matmul(bc[:], sel2[:], rd[:, ft, :],
                                 start=True, stop=True)
                nc.vector.tensor_tensor(oT[:, ft, :], o_raw[:], bc[:], op=OP.mult)

            for tt in range(TS // 128):
                ys = []
                for nb in range(2):
                    y = psP.tile([128, 512], F32, tag="m")
                    for ft in range(NCT):
                        nc.tensor.matmul(
                            y[:], oT[:, ft, tt * 128:(tt + 1) * 128],
                            wo[:, ft, nb * 512:(nb + 1) * 512],
                            start=(ft == 0), stop=False)
                    nc.tensor.matmul(y[:], ones1[:], bo[:, nb * 512:(nb + 1) * 512],
                                     start=False, stop=True)
                    ys.append(y)
                # LayerNorm over the 1024 features (two 512 psum halves)
                s0 = lnp.tile([128, 1], F32, tag="s0")
                s1 = lnp.tile([128, 1], F32, tag="s1")
                nc.vector.tensor_reduce(s0[:], ys[0][:], axis=mybir.AxisListType.X, op=OP.add)
                nc.vector.tensor_reduce(s1[:], ys[1][:], axis=mybir.AxisListType.X, op=OP.add)
                q0 = lnp.tile([128, 1], F32, tag="q0")
                q1 = lnp.tile([128, 1], F32, tag="q1")
                sq0 = lnp.tile([128, 512], F32, tag="sq0")
                sq1 = lnp.tile([128, 512], F32, tag="sq1")
                nc.scalar.activation(sq0[:], ys[0][:], AF.Square, accum_out=q0[:])
                nc.scalar.activation(sq1[:], ys[1][:], AF.Square, accum_out=q1[:])
                mu = lnp.tile([128, 1], F32, tag="mu")
                nc.vector.tensor_tensor(mu[:], s0[:], s1[:], op=OP.add)
                nc.vector.tensor_scalar_mul(mu[:], mu[:], 1.0 / C)
                var = lnp.tile([128, 1], F32, tag="var")
                nc.vector.tensor_tensor(var[:], q0[:], q1[:], op=OP.add)
                nc.vector.tensor_scalar_mul(var[:], var[:], 1.0 / C)
                m2 = lnp.tile([128, 1], F32, tag="m2")
                nc.vector.tensor_tensor(m2[:], mu[:], mu[:], op=OP.mult)
                nc.vector.tensor_tensor(var[:], var[:], m2[:], op=OP.subtract)
                sd = lnp.tile([128, 1], F32, tag="sd")
                nc.scalar.activation(sd[:], var[:], AF.Sqrt, bias=eps_t[:])
                istd = lnp.tile([128, 1], F32, tag="istd")
                nc.vector.reciprocal(istd[:], sd[:])
                yn = lnp.tile([128, C], F32, tag="yn")
                for nb in range(2):
                    nc.vector.tensor_scalar(
                        yn[:, nb * 512:(nb + 1) * 512], ys[nb][:],
                        mu[:], istd[:], op0=OP.subtract, op1=OP.mult)
                yf = lnp.tile([128, C], F32, tag="yf")
                nc.gpsimd.tensor_tensor(yf[:], yn[:], gam[:], op=OP.mult)
                nc.gpsimd.tensor_tensor(yf[:], yf[:], bet[:], op=OP.add)
                nc.sync.dma_start(out_h[tt * 128:(tt + 1) * 128, :], yf[:])

    nc.compile()
    return nc


def _get_nc():
    if "nc" not in _CACHE:
        _CACHE["nc"] = _build()
    return _CACHE["nc"]


def _make_in_maps(inputs):
    x = np.asarray(inputs["x"], np.float32)
    Wq = np.asarray(inputs["Wq"], np.float32)
    Wk = np.asarray(inputs["Wk"], np.float32)
    Wv = np.asarray(inputs["Wv"], np.float32)
    Wo = np.asarray(inputs["Wo"], np.float32)
    bq = np.asarray(inputs["bq"], np.float32)
    bk = np.asarray(inputs["bk"], np.float32)
    bv = np.asarray(inputs["bv"], np.float32)
    bo = np.asarray(inputs["bo"], np.float32)
    gamma = np.asarray(inputs["gamma"], np.float32)
    beta = np.asarray(inputs["beta"], np.float32)

    xT = np.ascontiguousarray(x.transpose(2, 0, 1)).astype(NP_X)  # [C, B, T]
    wo_c = np.ascontiguousarray(Wo).astype(NP_W)
    bo_row = np.ascontiguousarray(bo.reshape(1, C))
    gamb = np.ascontiguousarray(np.broadcast_to(gamma, (128, C)))
    betb = np.ascontiguousarray(np.broadcast_to(beta, (128, C)))

    maps = []
    for i in range(NCORES):
        cols = slice(DPC * i, DPC * (i + 1))
        bvb = np.zeros((128, HPC, 65), np.float32)
        bv_sh = bv[cols]
        for hh in range(HPC):
            bvb[:, hh, :64] = bv_sh[hh * 64:(hh + 1) * 64]
        maps.append({
            "xT": xT,
            "wq": np.ascontiguousarray(Wq[:, cols]).astype(NP_W),
            "wk": np.ascontiguousarray(Wk[:, cols]).astype(NP_W),
            "wv": np.ascontiguousarray(Wv[:, cols]).astype(NP_W),
            "wo": wo_c,
            "bqT": np.ascontiguousarray(bq[cols].reshape(DPC, 1)),
            "bkT": np.ascontiguousarray(bk[cols].reshape(DPC, 1)),
            "bvb": bvb,
            "bo_row": bo_row,
            "gamb": gamb,
            "betb": betb,
        })
    return maps


def _run(inputs, trace=False, **kwargs):
    nc = _get_nc()
    in_maps = _make_in_maps(inputs)
    res = run_bass_kernel_spmd(nc, in_maps, core_ids=list(range(NCORES)),
                               trace=trace, **kwargs)
    y = np.empty((B, T, C), np.float32)
    for i in range(NCORES):
        b, ts = divmod(i, 4)
        y[b, ts * TS:(ts + 1) * TS, :] = res.results[i]["out"]
    return y, res


def kernel(**inputs) -> np.ndarray:
    out, _ = _run(inputs, trace=False)
    return out


# revision 10
# speedup vs baseline: 1.0779x; 1.0779x over previous
"""Distributed Trainium2 Bass kernel for a causal attention block + LayerNorm.

Reference computation (B=2, T=2048, C=1024, H=16 heads, Dh=64):
    q,k,v = x@Wq+bq, x@Wk+bk, x@Wv+bv          (per-head split)
    att   = softmax(causal(q k^T / sqrt(Dh)))
    o     = att @ v ; y = o@Wo + bo ; out = LayerNorm(y) * gamma + beta

Sharding (8 cores, one TRN2 chip):
    Tensor-parallel over heads: core i owns heads {2i, 2i+1} for BOTH
    batches (Megatron-style column shards of Wq/Wk/Wv).  After attention,
    two 8-core AllToAlls (one per local head, bf16 payload) redistribute the
    per-head outputs (plus softmax denominators) to token-sharding: core i
    ends with tokens [b = i//4, t in (i%4)*512 ...] with ALL 1024 features,
    applies the softmax division, output projection (full Wo), bias and
    LayerNorm locally, and writes its (512, 1024) slice of the output.
    The first AllToAll overlaps the second head's attention compute.

Layout choices (all on-chip matmuls contract over the partition axis):
    - activations are feature-major: host passes x^T [C, B, T].
    - q^T,k^T,v^T [d, t] produced directly; v transposed on the PE into
      s-major v-hat [s, d] with an extra ones column per head so the P@V
      matmul also yields the softmax denominator for free.
    - scores are computed transposed: S^T[s, q] = k^T.T @ q^T, exp on the
      scalar engine straight out of PSUM (scale=1/8 folded in), causal
      masking via gpsimd.affine_select on the 128x128 diagonal tiles only,
      and P@V uses v-hat as the stationary operand so the unnormalized
      attention output O^T [d, q] is produced feature-major.
    - reciprocal of the softmax denominator is exp(-ln(d)) on the scalar
      engine (accurate, and the vector-engine reciprocal is very slow on
      a 2-partition tile), broadcast across partitions with a K=2 matmul.
"""

import os
import numpy as np
import ml_dtypes

import concourse.bass as bass
import concourse.mybir as mybir
import concourse.tile as tile
from concourse import bacc
from concourse.bass_utils import run_bass_kernel_spmd

F32 = mybir.dt.float32
BF16 = mybir.dt.bfloat16
AF = mybir.ActivationFunctionType
OP = mybir.AluOpType

B, T, C, H, Dh = 2, 2048, 1024, 16, 64
NCORES = 8
HPC = 2               # heads per core
DPC = HPC * Dh        # 128 feature columns per core
TS = 512              # output token-slice length per core
NQB = T // 512        # 4 q blocks
NST = T // 128        # 16 s tiles
NCT = C // 128        # 8 contraction tiles
EPS = 1e-5

DT_X = BF16
DT_W = BF16
DT_P = BF16
DT_A2A = BF16         # AllToAll payload dtype
NP_X = ml_dtypes.bfloat16
NP_W = ml_dtypes.bfloat16

_CACHE = {}


def _build():
    nc = bacc.Bacc("TRN2", target_bir_lowering=False, debug=False,
                   num_devices=NCORES)

    xT_h = nc.dram_tensor("xT", [C, B, T], DT_X, kind="ExternalInput")
    wq_h = nc.dram_tensor("wq", [C, DPC], DT_W, kind="ExternalInput")
    wk_h = nc.dram_tensor("wk", [C, DPC], DT_W, kind="ExternalInput")
    wv_h = nc.dram_tensor("wv", [C, DPC], DT_W, kind="ExternalInput")
    wo_h = nc.dram_tensor("wo", [C, C], DT_W, kind="ExternalInput")
    bqT_h = nc.dram_tensor("bqT", [DPC, 1], F32, kind="ExternalInput")
    bkT_h = nc.dram_tensor("bkT", [DPC, 1], F32, kind="ExternalInput")
    bvT_h = nc.dram_tensor("bvT", [DPC, 1], F32, kind="ExternalInput")
    bo_h = nc.dram_tensor("bo_row", [1, C], F32, kind="ExternalInput")
    gam_h = nc.dram_tensor("gamb", [128, C], F32, kind="ExternalInput")
    bet_h = nc.dram_tensor("betb", [128, C], F32, kind="ExternalInput")
    out_h = nc.dram_tensor("out", [TS, C], F32, kind="ExternalOutput")

    sel2_np = np.zeros((2, 128), np.float32)
    sel2_np[0, :64] = 1.0
    sel2_np[1, 64:] = 1.0
    sel2_d = nc.inline_tensor(sel2_np, name="sel2_const")
    ones1_d = nc.inline_tensor(np.ones((1, 128), np.float32), name="ones1_const")
    ident_d = nc.inline_tensor(
        np.eye(128, dtype=ml_dtypes.bfloat16), name="ident_const")

    with tile.TileContext(nc) as tc:
        with (
            tc.tile_pool(name="const", bufs=1) as cp,
            tc.tile_pool(name="dram", bufs=1, space="DRAM") as dp,
            tc.tile_pool(name="act", bufs=1) as ap,
            tc.tile_pool(name="pp", bufs=6) as pp,
            tc.tile_pool(name="vtp", bufs=2) as vtp,
            tc.tile_pool(name="ohp", bufs=3) as ohp,
            tc.tile_pool(name="orp", bufs=3) as orp,
            tc.tile_pool(name="psP", bufs=4, space="PSUM") as psP,
            tc.tile_pool(name="psV", bufs=2, space="PSUM") as psV,
            tc.tile_pool(name="psO", bufs=2, space="PSUM") as psO,
        ):
            # ---- constants ----
            sel2 = cp.tile([2, 128], F32)
            nc.sync.dma_start(sel2[:], sel2_d[:])
            ones1 = cp.tile([1, 128], F32)
            nc.sync.dma_start(ones1[:], ones1_d[:])
            ident = cp.tile([128, 128], BF16)
            nc.sync.dma_start(ident[:], ident_d[:])
            bqT = cp.tile([DPC, 1], F32)
            nc.sync.dma_start(bqT[:], bqT_h[:])
            bkT = cp.tile([DPC, 1], F32)
            nc.sync.dma_start(bkT[:], bkT_h[:])
            bvT = cp.tile([DPC, 1], F32)
            nc.sync.dma_start(bvT[:], bvT_h[:])
            bo = cp.tile([1, C], F32)
            nc.sync.dma_start(bo[:], bo_h[:])
            gam = cp.tile([128, C], F32)
            nc.sync.dma_start(gam[:], gam_h[:])
            bet = cp.tile([128, C], F32)
            nc.sync.dma_start(bet[:], bet_h[:])
            eps_t = cp.tile([128, 1], F32)
            nc.gpsimd.memset(eps_t[:], EPS)

            a2a_in0 = dp.tile([NCORES, 65, 512], DT_A2A, tag="ai0")
            a2a_in1 = dp.tile([NCORES, 65, 512], DT_A2A, tag="ai1")
            a2a_out0 = dp.tile([NCORES, 65, 512], DT_A2A, tag="ao0")
            a2a_out1 = dp.tile([NCORES, 65, 512], DT_A2A, tag="ao1")
            a2a_in = [a2a_in0, a2a_in1]
            a2a_out = [a2a_out0, a2a_out1]

            # ---- persistent activation tiles ----
            qT = ap.tile([DPC, B, T], DT_P)
            kT = ap.tile([DPC, B, T], DT_P)
            vhat = ap.tile([128, B, NST, HPC, 65], DT_P)
            oT = ap.tile([128, NCT, 512], DT_P)
            wo = ap.tile([128, NCT, C], DT_W)
            nc.sync.dma_start(
                wo[:], wo_h[:].rearrange("(ct p) m -> p ct m", p=128))
            for b in range(B):
                nc.gpsimd.memset(vhat[:, b, :, :, 64:65], 1.0)

            # ---- load x^T and weights (pool scoped to phase 1) ----
            xw_ctx = tc.tile_pool(name="xw", bufs=1)
            xw = xw_ctx.__enter__()
            xT = xw.tile([128, NCT, B, T], DT_X)
            xsrc = xT_h[:].rearrange("(ct p) b t -> p ct b t", p=128)
            for ct in range(NCT):
                nc.sync.dma_start(xT[:, ct], xsrc[:, ct])
            wq = xw.tile([128, NCT, DPC], DT_W)
            wk = xw.tile([128, NCT, DPC], DT_W)
            wv = xw.tile([128, NCT, DPC], DT_W)
            for w_sb, w_h in ((wq, wq_h), (wk, wk_h), (wv, wv_h)):
                nc.sync.dma_start(
                    w_sb[:], w_h[:].rearrange("(ct p) m -> p ct m", p=128))

            # ---- phase 1: projections, all feature-major [d, t] ----
            for b in range(B):
                for qb in range(NQB):
                    for w_sb, bias, dst in ((wq, bqT, qT), (wk, bkT, kT)):
                        ps = psP.tile([128, 512], F32, tag="m")
                        for ct in range(NCT):
                            nc.tensor.matmul(
                                ps[:], w_sb[:, ct], xT[:, ct, b, qb * 512:(qb + 1) * 512],
                                start=(ct == 0), stop=(ct == NCT - 1))
                        nc.vector.tensor_scalar_add(
                            dst[:, b, qb * 512:(qb + 1) * 512], ps[:], bias[:])
                    # v^T, then transpose 128x128 blocks into s-major vhat
                    ps = psP.tile([128, 512], F32, tag="m")
                    for ct in range(NCT):
                        nc.tensor.matmul(
                            ps[:], wv[:, ct], xT[:, ct, b, qb * 512:(qb + 1) * 512],
                            start=(ct == 0), stop=(ct == NCT - 1))
                    vt = vtp.tile([128, 512], DT_P, tag="vt")
                    nc.vector.tensor_scalar_add(vt[:], ps[:], bvT[:])
                    for sub in range(4):
                        st = qb * 4 + sub
                        tr = psV.tile([128, 128], DT_P, tag="v")
                        nc.tensor.transpose(
                            tr[:], vt[:, sub * 128:(sub + 1) * 128], ident[:])
                        nc.vector.tensor_copy(
                            vhat[:, b, st, :, 0:64],
                            tr[:].rearrange("p (hh d) -> p hh d", hh=HPC))

            xw_ctx.__exit__(None, None, None)

            # ---- phase 2 + 3: attention per local head, A2A after each ----
            for hh in range(HPC):
                hlo = hh * 64
                for b in range(B):
                    for qb in range(NQB):
                        o_ps = psO.tile([65, 512], F32, tag="o")
                        nsi = 4 * qb + 4
                        for si in range(nsi):
                            off = si * 128 - qb * 512
                            lo = max(off, 0)
                            w = 512 - lo
                            s_ps = psP.tile([128, w], F32, tag="m")
                            nc.tensor.matmul(
                                s_ps[:],
                                kT[hlo:hlo + 64, b, si * 128:(si + 1) * 128],
                                qT[hlo:hlo + 64, b, qb * 512 + lo:(qb + 1) * 512],
                                start=True, stop=True)
                            p_sb = pp.tile([128, w], DT_P, tag="p")
                            nc.scalar.activation(p_sb[:], s_ps[:], AF.Exp, scale=0.125)
                            if off >= 0:
                                # causal triangle on the 128-wide diagonal block
                                nc.gpsimd.affine_select(
                                    out=p_sb[:, 0:128], in_=p_sb[:, 0:128],
                                    compare_op=OP.is_ge, fill=0.0, base=0,
                                    channel_multiplier=-1, pattern=[[1, 128]])
                            nc.tensor.matmul(
                                o_ps[:, lo:512], vhat[:, b, si, hh, :], p_sb[:],
                                start=(si == 0), stop=(si == nsi - 1))
                        oh = ohp.tile([65, 512], DT_A2A, tag="oh")
                        nc.vector.tensor_copy(oh[:], o_ps[:])
                        nc.sync.dma_start(a2a_in[hh][b * 4 + qb, :, :], oh[:])
                nc.gpsimd.collective_compute(
                    "AllToAll", OP.bypass,
                    replica_groups=[list(range(NCORES))],
                    ins=[a2a_in[hh].opt()], outs=[a2a_out[hh].opt()])

            # ---- phase 4: softmax division, out-proj, LayerNorm ----
            # denom[j, a, n] = softmax denominator of head 2*a+j at token n
            denom = cp.tile([2, NCORES, 512], DT_A2A)
            for j in range(2):
                nc.sync.dma_start(denom[j:j + 1, :, :], a2a_out[j][:, 64, :])
            # reciprocal via exp(-ln(d)) on the scalar engine (accurate; the
            # vector-engine reciprocal runs on only 2 partitions here = slow)
            rd = cp.tile([2, NCORES, 512], F32)
            nc.scalar.activation(rd[:], denom[:], AF.Ln)
            nc.scalar.activation(rd[:], rd[:], AF.Exp, scale=-1.0)

            lnp_ctx = tc.tile_pool(name="lnp", bufs=2)
            lnp = lnp_ctx.__enter__()
            for ft in range(NCT):
                o_raw = orp.tile([128, 512], DT_A2A, tag="oraw")
                for j in range(2):
                    nc.sync.dma_start(
                        o_raw[j * 64:(j + 1) * 64, :],
                        a2a_out[j][ft, 0:64, :])
                bc = psP.tile([128, 512], F32, tag="m")
                nc.tensor.matmul(bc[:], sel2[:], rd[:, ft, :],
                                 start=True, stop=True)
                nc.vector.tensor_tensor(oT[:, ft, :], o_raw[:], bc[:], op=OP.mult)

            for tt in range(TS // 128):
                ys = []
                for nb in range(2):
                    y = psP.tile([128, 512], F32, tag="m")
                    for ft in range(NCT):
                        nc.tensor.matmul(
                            y[:], oT[:, ft, tt * 128:(tt + 1) * 128],
                            wo[:, ft, nb * 512:(nb + 1) * 512],
                            start=(ft == 0), stop=False)
                    nc.tensor.matmul(y[:], ones1[:], bo[:, nb * 512:(nb + 1) * 512],
                                     start=False, stop=True)
                    ys.append(y)
                # LayerNorm over the 1024 features (two 512 psum halves)
                s0 = lnp.tile([128, 1], F32, tag="s0")
                s1 = lnp.tile([128, 1], F32, tag="s1")
                nc.vector.tensor_reduce(s0[:], ys[0][:], axis=mybir.AxisListType.X, op=OP.add)
                nc.vector.tensor_reduce(s1[:], ys[1][:], axis=mybir.AxisListType.X, op=OP.add)
                q0 = lnp.tile([128, 1], F32, tag="q0")
                q1 = lnp.tile([128, 1], F32, tag="q1")
                sq0 = lnp.tile([128, 512], F32, tag="sq0")
                sq1 = lnp.tile([128, 512], F32, tag="sq1")
                nc.scalar.activation(sq0[:], ys[0][:], AF.Square, accum_out=q0[:])
                nc.scalar.activation(sq1[:], ys[1][:], AF.Square, accum_out=q1[:])
                mu = lnp.tile([128, 1], F32, tag="mu")
                nc.vector.tensor_tensor(mu[:], s0[:], s1[:], op=OP.add)
                nc.vector.tensor_scalar_mul(mu[:], mu[:], 1.0 / C)
                var = lnp.tile([128, 1], F32, tag="var")
                nc.vector.tensor_tensor(var[:], q0[:], q1[:], op=OP.add)
                nc.vector.tensor_scalar_mul(var[:], var[:], 1.0 / C)
                m2 = lnp.tile([128, 1], F32, tag="m2")
                nc.vector.tensor_tensor(m2[:], mu[:], mu[:], op=OP.mult)
                nc.vector.tensor_tensor(var[:], var[:], m2[:], op=OP.subtract)
                sd = lnp.tile([128, 1], F32, tag="sd")
                nc.scalar.activation(sd[:], var[:], AF.Sqrt, bias=eps_t[:])
                istd = lnp.tile([128, 1], F32, tag="istd")
                nc.vector.reciprocal(istd[:], sd[:])
                yn = lnp.tile([128, C], F32, tag="yn")
                for nb in range(2):
                    nc.vector.tensor_scalar(
                        yn[:, nb * 512:(nb + 1) * 512], ys[nb][:],
                        mu[:], istd[:], op0=OP.subtract, op1=OP.mult)
                yf = lnp.tile([128, C], F32, tag="yf")
                nc.gpsimd.tensor_tensor(yf[:], yn[:], gam[:], op=OP.mult)
                nc.gpsimd.tensor_tensor(yf[:], yf[:], bet[:], op=OP.add)
                nc.sync.dma_start(out_h[tt * 128:(tt + 1) * 128, :], yf[:])
            lnp_ctx.__exit__(None, None, None)

    nc.compile()
    return nc


def _get_nc():
    if "nc" not in _CACHE:
        _CACHE["nc"] = _build()
    return _CACHE["nc"]


def _make_in_maps(inputs):
    x = np.asarray(inputs["x"], np.float32)
    Wq = np.asarray(inputs["Wq"], np.float32)
    Wk = np.asarray(inputs["Wk"], np.float32)
    Wv = np.asarray(inputs["Wv"], np.float32)
    Wo = np.asarray(inputs["Wo"], np.float32)
    bq = np.asarray(inputs["bq"], np.float32)
    bk = np.asarray(inputs["bk"], np.float32)
    bv = np.asarray(inputs["bv"], np.float32)
    bo = np.asarray(inputs["bo"], np.float32)
    gamma = np.asarray(inputs["gamma"], np.float32)
    beta = np.asarray(inputs["beta"], np.float32)

    xT = np.ascontiguousarray(x.transpose(2, 0, 1)).astype(NP_X)  # [C, B, T]
    wo_c = np.ascontiguousarray(Wo).astype(NP_W)
    bo_row = np.ascontiguousarray(bo.reshape(1, C))
    gamb = np.ascontiguousarray(np.broadcast_to(gamma, (128, C)))
    betb = np.ascontiguousarray(np.broadcast_to(beta, (128, C)))

    maps = []
    for i in range(NCORES):
        cols = slice(DPC * i, DPC * (i + 1))
        maps.append({
            "xT": xT,
            "wq": np.ascontiguousarray(Wq[:, cols]).astype(NP_W),
            "wk": np.ascontiguousarray(Wk[:, cols]).astype(NP_W),
            "wv": np.ascontiguousarray(Wv[:, cols]).astype(NP_W),
            "wo": wo_c,
            "bqT": np.ascontiguousarray(bq[cols].reshape(DPC, 1)),
            "bkT": np.ascontiguousarray(bk[cols].reshape(DPC, 1)),
            "bvT": np.ascontiguousarray(bv[cols].reshape(DPC, 1)),
            "bo_row": bo_row,
            "gamb": gamb,
            "betb": betb,
        })
    return maps


def _run(inputs, trace=False, **kwargs):
    nc = _get_nc()
    in_maps = _make_in_maps(inputs)
    res = run_bass_kernel_spmd(nc, in_maps, core_ids=list(range(NCORES)),
                               trace=trace, **kwargs)
    y = np.empty((B, T, C), np.float32)
    for i in range(NCORES):
        b, ts = divmod(i, 4)
        y[b, ts * TS:(ts + 1) * TS, :] = res.results[i]["out"]
    return y, res


def kernel(**inputs) -> np.ndarray:
    out, _ = _run(inputs, trace=False)
    return out


# revision 15
# speedup vs baseline: 1.1009x; 1.0213x over previous
"""Distributed Trainium2 Bass kernel for a causal attention block + LayerNorm.

Reference computation (B=2, T=2048, C=1024, H=16 heads, Dh=64):
    q,k,v = x@Wq+bq, x@Wk+bk, x@Wv+bv          (per-head split)
    att   = softmax(causal(q k^T / sqrt(Dh)))
    o     = att @ v ; y = o@Wo + bo ; out = LayerNorm(y) * gamma + beta

Sharding (8 cores, one TRN2 chip):
    Tensor-parallel over heads: core i owns heads {2i, 2i+1} for BOTH
    batches (Megatron-style column shards of Wq/Wk/Wv).  After attention,
    two 8-core AllToAlls (one per local head, bf16 payload) redistribute the
    per-head outputs (plus softmax denominators) to token-sharding: core i
    ends with tokens [b = i//4, t in (i%4)*512 ...] with ALL 1024 features,
    applies the softmax division, output projection (full Wo), bias and
    LayerNorm locally, and writes its (512, 1024) slice of the output.

Schedule: projections are interleaved with head-0 attention per (b, q-block)
so the scalar engine (exp is the phase bound) starts almost immediately;
the first AllToAll overlaps head-1 attention; per-head output prep overlaps
the second AllToAll.

Layout choices (all on-chip matmuls contract over the partition axis):
    - activations are feature-major: host passes x^T [C, B, T].
    - q^T,k^T,v^T [d, t] produced directly; v transposed on the PE into
      s-major v-hat [s, d] with an extra ones column per head so the P@V
      matmul also yields the softmax denominator for free.
    - scores are computed transposed: S^T[s, q] = k^T.T @ q^T; score chunks
      are packed in pairs into 2-bank PSUM tiles so each scalar-engine Exp
      call covers up to 1024 columns (halves the per-call overhead); causal
      masking via gpsimd.affine_select on the 128-wide diagonal blocks only;
      P@V uses v-hat as the stationary operand so the unnormalized attention
      output O^T [d, q] is produced feature-major (no transposes needed).
    - reciprocal of the softmax denominator is exp(-ln(d)) on the scalar
      engine (accurate; the vector-engine reciprocal is very slow on a
      1-partition tile), broadcast across partitions with a K=1 matmul.
"""

import os
import numpy as np
import ml_dtypes

import concourse.bass as bass
import concourse.mybir as mybir
import concourse.tile as tile
from concourse import bacc
from concourse.bass_utils import run_bass_kernel_spmd

F32 = mybir.dt.float32
BF16 = mybir.dt.bfloat16
AF = mybir.ActivationFunctionType
OP = mybir.AluOpType

B, T, C, H, Dh = 2, 2048, 1024, 16, 64
NCORES = 8
HPC = 2               # heads per core
DPC = HPC * Dh        # 128 feature columns per core
TS = 512              # output token-slice length per core
NQB = T // 512        # 4 q blocks
NST = T // 128        # 16 s tiles
NCT = C // 128        # 8 contraction tiles
EPS = 1e-5

DT_X = BF16
DT_W = BF16
DT_P = BF16
DT_A2A = BF16         # AllToAll payload dtype
NP_X = ml_dtypes.bfloat16
NP_W = ml_dtypes.bfloat16

_CACHE = {}

V3_LN = os.environ.get("V3_LN", "1") == "1"
V3_PAIR = os.environ.get("V3_PAIR", "1") == "1"
V3_GDMA = os.environ.get("V3_GDMA", "1") == "1"
V3_PREP = os.environ.get("V3_PREP", "1") == "1"


def _build():
    nc = bacc.Bacc("TRN2", target_bir_lowering=False, debug=False,
                   num_devices=NCORES)

    xT_h = nc.dram_tensor("xT", [C, B, T], DT_X, kind="ExternalInput")
    wq_h = nc.dram_tensor("wq", [C, DPC], DT_W, kind="ExternalInput")
    wk_h = nc.dram_tensor("wk", [C, DPC], DT_W, kind="ExternalInput")
    wv_h = nc.dram_tensor("wv", [C, DPC], DT_W, kind="ExternalInput")
    wo_h = nc.dram_tensor("wo", [C, C], DT_W, kind="ExternalInput")
    bqT_h = nc.dram_tensor("bqT", [DPC, 1], F32, kind="ExternalInput")
    bkT_h = nc.dram_tensor("bkT", [DPC, 1], F32, kind="ExternalInput")
    bvT_h = nc.dram_tensor("bvT", [DPC, 1], F32, kind="ExternalInput")
    bo_h = nc.dram_tensor("bo_row", [1, C], F32, kind="ExternalInput")
    gam_h = nc.dram_tensor("gamb", [128, C], F32, kind="ExternalInput")
    bet_h = nc.dram_tensor("betb", [128, C], F32, kind="ExternalInput")
    out_h = nc.dram_tensor("out", [TS, C], F32, kind="ExternalOutput")

    ones1_d = nc.inline_tensor(np.ones((1, 128), np.float32), name="ones1_const")
    ident_d = nc.inline_tensor(
        np.eye(128, dtype=ml_dtypes.bfloat16), name="ident_const")

    with tile.TileContext(nc) as tc:
        with (
            tc.tile_pool(name="const", bufs=1) as cp,
            tc.tile_pool(name="dram", bufs=1, space="DRAM") as dp,
            tc.tile_pool(name="act", bufs=1) as ap,
            tc.tile_pool(name="pp", bufs=5) as pp,
            tc.tile_pool(name="vtp", bufs=2) as vtp,
            tc.tile_pool(name="ohp", bufs=3) as ohp,
            tc.tile_pool(name="orp", bufs=3) as orp,
            tc.tile_pool(name="psM", bufs=2, space="PSUM") as psM,
            tc.tile_pool(name="psS2", bufs=2, space="PSUM") as psS2,
            tc.tile_pool(name="psOC", bufs=2, space="PSUM") as psOC,
        ):
            # ---- small constants (issued on sync queue) ----
            bqT = cp.tile([DPC, 1], F32)
            nc.sync.dma_start(bqT[:], bqT_h[:])
            bkT = cp.tile([DPC, 1], F32)
            nc.sync.dma_start(bkT[:], bkT_h[:])
            bvT = cp.tile([DPC, 1], F32)
            nc.sync.dma_start(bvT[:], bvT_h[:])
            ident = cp.tile([128, 128], BF16)
            nc.sync.dma_start(ident[:], ident_d[:])
            ones1 = cp.tile([1, 128], F32)
            nc.sync.dma_start(ones1[:], ones1_d[:])

            # ---- weights first (small, needed immediately), x^T on the
            # gpsimd DMA queue in parallel ----
            xw_ctx = tc.tile_pool(name="xw", bufs=1)
            xw = xw_ctx.__enter__()
            wq = xw.tile([128, NCT, DPC], DT_W)
            wk = xw.tile([128, NCT, DPC], DT_W)
            wv = xw.tile([128, NCT, DPC], DT_W)
            for w_sb, w_h in ((wq, wq_h), (wk, wk_h), (wv, wv_h)):
                nc.sync.dma_start(
                    w_sb[:], w_h[:].rearrange("(ct p) m -> p ct m", p=128))
            xT = xw.tile([128, NCT, B, T], DT_X)
            xsrc = xT_h[:].rearrange("(ct p) b t -> p ct b t", p=128)
            for ct in range(NCT):
                (nc.gpsimd if V3_GDMA else nc.sync).dma_start(xT[:, ct], xsrc[:, ct])

            # ---- persistent activation tiles ----
            qT = ap.tile([DPC, B, T], DT_P)
            kT = ap.tile([DPC, B, T], DT_P)
            vhat = ap.tile([128, B, NST, HPC, 65], DT_P)
            oT = ap.tile([128, NCT, 512], DT_P)
            for b in range(B):
                nc.gpsimd.memset(vhat[:, b, :, :, 64:65], 1.0)

            a2a_in0 = dp.tile([NCORES, 65, 512], DT_A2A, tag="ai0")
            a2a_in1 = dp.tile([NCORES, 65, 512], DT_A2A, tag="ai1")
            a2a_out0 = dp.tile([NCORES, 65, 512], DT_A2A, tag="ao0")
            a2a_out1 = dp.tile([NCORES, 65, 512], DT_A2A, tag="ao1")
            a2a_in = [a2a_in0, a2a_in1]
            a2a_out = [a2a_out0, a2a_out1]

            def proj(b, qb):
                sl = slice(qb * 512, (qb + 1) * 512)
                for w_sb, bias, dst in ((wq, bqT, qT), (wk, bkT, kT)):
                    ps = psM.tile([128, 512], F32, tag="m")
                    for ct in range(NCT):
                        nc.tensor.matmul(ps[:], w_sb[:, ct], xT[:, ct, b, sl],
                                         start=(ct == 0), stop=(ct == NCT - 1))
                    nc.vector.tensor_scalar_add(dst[:, b, sl], ps[:], bias[:])
                # v^T, then transpose 128x128 blocks into s-major vhat
                ps = psM.tile([128, 512], F32, tag="m")
                for ct in range(NCT):
                    nc.tensor.matmul(ps[:], wv[:, ct], xT[:, ct, b, sl],
                                     start=(ct == 0), stop=(ct == NCT - 1))
                vt = vtp.tile([128, 512], DT_P, tag="vt")
                nc.vector.tensor_scalar_add(vt[:], ps[:], bvT[:])
                for sub in range(4):
                    st = qb * 4 + sub
                    tr = psM.tile([128, 128], DT_P, tag="m")
                    nc.tensor.transpose(
                        tr[:], vt[:, sub * 128:(sub + 1) * 128], ident[:])
                    nc.vector.tensor_copy(
                        vhat[:, b, st, :, 0:64],
                        tr[:].rearrange("p (hh d) -> p hh d", hh=HPC))

            def attn(hh, b, qb):
                hlo = hh * 64
                o_ps = psOC.tile([65, 512], F32, tag="o")
                nsi = 4 * qb + 4
                # chunks (si, lo): lo = in-block column offset; pack pairs
                # into one 2-bank PSUM tile so exp covers both
                chunks = [(si, 0) for si in range(4 * qb)] + \
                         [(si, si * 128 - qb * 512) for si in range(4 * qb, nsi)]
                groups = []
                i = 0
                while i < len(chunks):
                    w0 = 512 - chunks[i][1]
                    if V3_PAIR and i + 1 < len(chunks) and w0 + (512 - chunks[i + 1][1]) <= 1024:
                        groups.append([chunks[i], chunks[i + 1]])
                        i += 2
                    else:
                        groups.append([chunks[i]])
                        i += 1
                for grp in groups:
                    tot = sum(512 - lo for _, lo in grp)
                    s_ps = psS2.tile([128, 1024], F32, tag="s2")
                    p_sb = pp.tile([128, 1024], DT_P, tag="p")
                    off = 0
                    for si, lo in grp:
                        w = 512 - lo
                        nc.tensor.matmul(
                            s_ps[:, off:off + w],
                            kT[hlo:hlo + 64, b, si * 128:(si + 1) * 128],
                            qT[hlo:hlo + 64, b, qb * 512 + lo:(qb + 1) * 512],
                            start=True, stop=True)
                        off += w
                    nc.scalar.activation(p_sb[:, 0:tot], s_ps[:, 0:tot],
                                         AF.Exp, scale=0.125)
                    off = 0
                    for si, lo in grp:
                        w = 512 - lo
                        if lo > 0 or si * 128 == qb * 512:
                            # diagonal block: causal triangle mask
                            nc.gpsimd.affine_select(
                                out=p_sb[:, off:off + 128],
                                in_=p_sb[:, off:off + 128],
                                compare_op=OP.is_ge, fill=0.0, base=0,
                                channel_multiplier=-1, pattern=[[1, 128]])
                        nc.tensor.matmul(
                            o_ps[:, lo:512], vhat[:, b, si, hh, :],
                            p_sb[:, off:off + w],
                            start=(si == 0), stop=(si == nsi - 1))
                        off += w
                oh = ohp.tile([65, 512], DT_A2A, tag="oh")
                nc.vector.tensor_copy(oh[:], o_ps[:])
                nc.sync.dma_start(a2a_in[hh][b * 4 + qb, :, :], oh[:])

            def half_prep(j):
                """after AllToAll j: reciprocal of denominators, broadcast,
                and scale this head-half of o^T (rows j*64..j*64+64)."""
                dnm = cp.tile([1, NCORES, 512], DT_A2A, tag=f"dnm{j}")
                nc.sync.dma_start(dnm[:], a2a_out[j][:, 64, :])
                rd = cp.tile([1, NCORES, 512], F32, tag=f"rd{j}")
                nc.scalar.activation(rd[:], dnm[:], AF.Ln)
                nc.scalar.activation(rd[:], rd[:], AF.Exp, scale=-1.0)
                for ft in range(NCT):
                    o_raw = orp.tile([64, 512], DT_A2A, tag="oraw")
                    nc.sync.dma_start(o_raw[:], a2a_out[j][ft, 0:64, :])
                    bch = psM.tile([64, 512], F32, tag="m")
                    nc.tensor.matmul(bch[:], ones1[:, 0:64], rd[0:1, ft, :],
                                     start=True, stop=True)
                    nc.vector.tensor_tensor(
                        oT[j * 64:(j + 1) * 64, ft, :], o_raw[:], bch[:],
                        op=OP.mult)

            # ---- phase 1+2 interleaved; A2A per head ----
            for b in range(B):
                for qb in range(NQB):
                    proj(b, qb)
                    attn(0, b, qb)
            nc.gpsimd.collective_compute(
                "AllToAll", OP.bypass, replica_groups=[list(range(NCORES))],
                ins=[a2a_in[0].opt()], outs=[a2a_out[0].opt()])
            half_prep(0)
            for b in range(B):
                for qb in range(NQB):
                    attn(1, b, qb)
            xw_ctx.__exit__(None, None, None)
            nc.gpsimd.collective_compute(
                "AllToAll", OP.bypass, replica_groups=[list(range(NCORES))],
                ins=[a2a_in[1].opt()], outs=[a2a_out[1].opt()])
            half_prep(1)

            # ---- phase 4: out-proj + bias + LayerNorm on my token slice ----
            lnp_ctx = tc.tile_pool(name="lnp", bufs=2)
            lnp = lnp_ctx.__enter__()
            wo = lnp.tile([128, NCT, C], DT_W, tag="wo")
            nc.gpsimd.dma_start(
                wo[:], wo_h[:].rearrange("(ct p) m -> p ct m", p=128))
            bo = lnp.tile([1, C], F32, tag="bo")
            nc.sync.dma_start(bo[:], bo_h[:])
            gam = lnp.tile([128, C], F32, tag="gam")
            nc.sync.dma_start(gam[:], gam_h[:])
            bet = lnp.tile([128, C], F32, tag="bet")
            nc.sync.dma_start(bet[:], bet_h[:])
            eps_t = lnp.tile([128, 1], F32, tag="eps")
            nc.gpsimd.memset(eps_t[:], EPS)

            for tt in range(TS // 128):
                yc = lnp.tile([128, C], F32, tag="yc")
                s0 = lnp.tile([128, 1], F32, tag="s0")
                s1 = lnp.tile([128, 1], F32, tag="s1")
                q0 = lnp.tile([128, 1], F32, tag="q0")
                q1 = lnp.tile([128, 1], F32, tag="q1")
                sq = lnp.tile([128, 512], F32, tag="sq")
                for nb, (s_acc, q_acc) in enumerate(((s0, q0), (s1, q1))):
                    y = psM.tile([128, 512], F32, tag="m")
                    for ft in range(NCT):
                        nc.tensor.matmul(
                            y[:], oT[:, ft, tt * 128:(tt + 1) * 128],
                            wo[:, ft, nb * 512:(nb + 1) * 512],
                            start=(ft == 0), stop=False)
                    nc.tensor.matmul(y[:], ones1[:], bo[:, nb * 512:(nb + 1) * 512],
                                     start=False, stop=True)
                    half = slice(nb * 512, (nb + 1) * 512)
                    if V3_LN:
                        # move to SBUF + row-sum on the scalar engine
                        nc.scalar.activation(yc[:, half], y[:], AF.Identity,
                                             accum_out=s_acc[:])
                        # sum of squares on the scalar engine
                        nc.scalar.activation(sq[:], y[:], AF.Square,
                                             accum_out=q_acc[:])
                    else:
                        nc.vector.tensor_reduce(
                            s_acc[:], y[:], axis=mybir.AxisListType.X, op=OP.add)
                        nc.scalar.activation(sq[:], y[:], AF.Square,
                                             accum_out=q_acc[:])
                        nc.vector.tensor_copy(yc[:, half], y[:])
                mu = lnp.tile([128, 1], F32, tag="mu")
                nc.vector.tensor_tensor(mu[:], s0[:], s1[:], op=OP.add)
                nc.vector.tensor_scalar_mul(mu[:], mu[:], 1.0 / C)
                var = lnp.tile([128, 1], F32, tag="var")
                nc.vector.tensor_tensor(var[:], q0[:], q1[:], op=OP.add)
                nc.vector.tensor_scalar_mul(var[:], var[:], 1.0 / C)
                m2 = lnp.tile([128, 1], F32, tag="m2")
                nc.vector.tensor_tensor(m2[:], mu[:], mu[:], op=OP.mult)
                nc.vector.tensor_tensor(var[:], var[:], m2[:], op=OP.subtract)
                sd = lnp.tile([128, 1], F32, tag="sd")
                nc.scalar.activation(sd[:], var[:], AF.Sqrt, bias=eps_t[:])
                istd = lnp.tile([128, 1], F32, tag="istd")
                nc.vector.reciprocal(istd[:], sd[:])
                yn = lnp.tile([128, C], F32, tag="yn")
                nc.vector.tensor_scalar(
                    yn[:], yc[:], mu[:], istd[:], op0=OP.subtract, op1=OP.mult)
                yf = lnp.tile([128, C], F32, tag="yf")
                nc.gpsimd.tensor_tensor(yf[:], yn[:], gam[:], op=OP.mult)
                nc.gpsimd.tensor_tensor(yf[:], yf[:], bet[:], op=OP.add)
                nc.sync.dma_start(out_h[tt * 128:(tt + 1) * 128, :], yf[:])
            lnp_ctx.__exit__(None, None, None)

    nc.compile()
    return nc


def _get_nc():
    if "nc" not in _CACHE:
        _CACHE["nc"] = _build()
    return _CACHE["nc"]


def _make_in_maps(inputs):
    x = np.asarray(inputs["x"], np.float32)
    Wq = np.asarray(inputs["Wq"], np.float32)
    Wk = np.asarray(inputs["Wk"], np.float32)
    Wv = np.asarray(inputs["Wv"], np.float32)
    Wo = np.asarray(inputs["Wo"], np.float32)
    bq = np.asarray(inputs["bq"], np.float32)
    bk = np.asarray(inputs["bk"], np.float32)
    bv = np.asarray(inputs["bv"], np.float32)
    bo = np.asarray(inputs["bo"], np.float32)
    gamma = np.asarray(inputs["gamma"], np.float32)
    beta = np.asarray(inputs["beta"], np.float32)

    xT = np.ascontiguousarray(x.transpose(2, 0, 1)).astype(NP_X)  # [C, B, T]
    wo_c = np.ascontiguousarray(Wo).astype(NP_W)
    bo_row = np.ascontiguousarray(bo.reshape(1, C))
    gamb = np.ascontiguousarray(np.broadcast_to(gamma, (128, C)))
    betb = np.ascontiguousarray(np.broadcast_to(beta, (128, C)))

    maps = []
    for i in range(NCORES):
        cols = slice(DPC * i, DPC * (i + 1))
        maps.append({
            "xT": xT,
            "wq": np.ascontiguousarray(Wq[:, cols]).astype(NP_W),
            "wk": np.ascontiguousarray(Wk[:, cols]).astype(NP_W),
            "wv": np.ascontiguousarray(Wv[:, cols]).astype(NP_W),
            "wo": wo_c,
            "bqT": np.ascontiguousarray(bq[cols].reshape(DPC, 1)),
            "bkT": np.ascontiguousarray(bk[cols].reshape(DPC, 1)),
            "bvT": np.ascontiguousarray(bv[cols].reshape(DPC, 1)),
            "bo_row": bo_row,
            "gamb": gamb,
            "betb": betb,
        })
    return maps


def _run(inputs, trace=False, **kwargs):
    nc = _get_nc()
    in_maps = _make_in_maps(inputs)
    res = run_bass_kernel_spmd(nc, in_maps, core_ids=list(range(NCORES)),
                               trace=trace, **kwargs)
    y = np.empty((B, T, C), np.float32)
    for i in range(NCORES):
        b, ts = divmod(i, 4)
        y[b, ts * TS:(ts + 1) * TS, :] = res.results[i]["out"]
    return y, res


def kernel(**inputs) -> np.ndarray:
    out, _ = _run(inputs, trace=False)
    return out


# revision 16
# speedup vs baseline: 1.1252x; 1.0221x over previous
"""Distributed Trainium2 Bass kernel for a causal attention block + LayerNorm.

Reference computation (B=2, T=2048, C=1024, H=16 heads, Dh=64):
    q,k,v = x@Wq+bq, x@Wk+bk, x@Wv+bv          (per-head split)
    att   = softmax(causal(q k^T / sqrt(Dh)))
    o     = att @ v ; y = o@Wo + bo ; out = LayerNorm(y) * gamma + beta

Sharding (8 cores, one TRN2 chip):
    Tensor-parallel over heads: core i owns heads {2i, 2i+1} for BOTH
    batches (Megatron-style column shards of Wq/Wk/Wv).  After attention,
    two 8-core AllToAlls (one per local head, bf16 payload) redistribute the
    per-head outputs (plus softmax denominators) to token-sharding: core i
    ends with tokens [b = i//4, t in (i%4)*512 ...] with ALL 1024 features,
    applies the softmax division, output projection (full Wo), bias and
    LayerNorm locally, and writes its (512, 1024) slice of the output.

Schedule: projections are interleaved with head-0 attention per (b, q-block)
so the scalar engine (exp is the phase bound) starts almost immediately;
the first AllToAll overlaps head-1 attention; per-head output prep overlaps
the second AllToAll.

Layout choices (all on-chip matmuls contract over the partition axis):
    - activations are feature-major: host passes x^T [C, B, T].
    - q^T,k^T,v^T [d, t] produced directly; v transposed on the PE into
      s-major v-hat [s, d] with an extra ones column per head so the P@V
      matmul also yields the softmax denominator for free.
    - scores are computed transposed: S^T[s, q] = k^T.T @ q^T; score chunks
      are packed in pairs into 2-bank PSUM tiles so each scalar-engine Exp
      call covers up to 1024 columns (halves the per-call overhead); causal
      masking via gpsimd.affine_select on the 128-wide diagonal blocks only;
      P@V uses v-hat as the stationary operand so the unnormalized attention
      output O^T [d, q] is produced feature-major (no transposes needed).
    - reciprocal of the softmax denominator is exp(-ln(d)) on the scalar
      engine (accurate; the vector-engine reciprocal is very slow on a
      1-partition tile), broadcast across partitions with a K=1 matmul.
"""

import os
import numpy as np
import ml_dtypes

import concourse.bass as bass
import concourse.mybir as mybir
import concourse.tile as tile
from concourse import bacc
from concourse.bass_utils import run_bass_kernel_spmd

F32 = mybir.dt.float32
BF16 = mybir.dt.bfloat16
AF = mybir.ActivationFunctionType
OP = mybir.AluOpType

B, T, C, H, Dh = 2, 2048, 1024, 16, 64
NCORES = 8
HPC = 2               # heads per core
DPC = HPC * Dh        # 128 feature columns per core
TS = 512              # output token-slice length per core
NQB = T // 512        # 4 q blocks
NST = T // 128        # 16 s tiles
NCT = C // 128        # 8 contraction tiles
EPS = 1e-5

DT_X = BF16
DT_W = BF16
DT_P = BF16
DT_A2A = BF16         # AllToAll payload dtype
NP_X = ml_dtypes.bfloat16
NP_W = ml_dtypes.bfloat16

_CACHE = {}

V3_LN = os.environ.get("V3_LN", "1") == "1"
V3_PAIR = os.environ.get("V3_PAIR", "1") == "1"
V3_GDMA = os.environ.get("V3_GDMA", "1") == "1"
V3_PREP = os.environ.get("V3_PREP", "1") == "1"


def _build():
    nc = bacc.Bacc("TRN2", target_bir_lowering=False, debug=False,
                   num_devices=NCORES)

    xT_h = nc.dram_tensor("xT", [C, B, T], DT_X, kind="ExternalInput")
    wq_h = nc.dram_tensor("wq", [C, DPC], DT_W, kind="ExternalInput")
    wk_h = nc.dram_tensor("wk", [C, DPC], DT_W, kind="ExternalInput")
    wv_h = nc.dram_tensor("wv", [C, DPC], DT_W, kind="ExternalInput")
    wo_h = nc.dram_tensor("wo", [C, C], DT_W, kind="ExternalInput")
    bqT_h = nc.dram_tensor("bqT", [DPC, 1], F32, kind="ExternalInput")
    bkT_h = nc.dram_tensor("bkT", [DPC, 1], F32, kind="ExternalInput")
    bvT_h = nc.dram_tensor("bvT", [DPC, 1], F32, kind="ExternalInput")
    bo_h = nc.dram_tensor("bo_row", [1, C], BF16, kind="ExternalInput")
    gam_h = nc.dram_tensor("gamb", [128, C], BF16, kind="ExternalInput")
    bet_h = nc.dram_tensor("betb", [128, C], BF16, kind="ExternalInput")
    out_h = nc.dram_tensor("out", [TS, C], BF16, kind="ExternalOutput")

    ones1_d = nc.inline_tensor(np.ones((1, 128), ml_dtypes.bfloat16), name="ones1_const")
    ident_d = nc.inline_tensor(
        np.eye(128, dtype=ml_dtypes.bfloat16), name="ident_const")

    with tile.TileContext(nc) as tc:
        with (
            tc.tile_pool(name="const", bufs=1) as cp,
            tc.tile_pool(name="dram", bufs=1, space="DRAM") as dp,
            tc.tile_pool(name="act", bufs=1) as ap,
            tc.tile_pool(name="pp", bufs=5) as pp,
            tc.tile_pool(name="vtp", bufs=2) as vtp,
            tc.tile_pool(name="ohp", bufs=3) as ohp,
            tc.tile_pool(name="orp", bufs=3) as orp,
            tc.tile_pool(name="psM", bufs=2, space="PSUM") as psM,
            tc.tile_pool(name="psS2", bufs=2, space="PSUM") as psS2,
            tc.tile_pool(name="psOC", bufs=2, space="PSUM") as psOC,
        ):
            # ---- small constants (issued on sync queue) ----
            bqT = cp.tile([DPC, 1], F32)
            nc.sync.dma_start(bqT[:], bqT_h[:])
            bkT = cp.tile([DPC, 1], F32)
            nc.sync.dma_start(bkT[:], bkT_h[:])
            bvT = cp.tile([DPC, 1], F32)
            nc.sync.dma_start(bvT[:], bvT_h[:])
            ident = cp.tile([128, 128], BF16)
            nc.sync.dma_start(ident[:], ident_d[:])
            ones1 = cp.tile([1, 128], BF16)
            nc.sync.dma_start(ones1[:], ones1_d[:])

            # ---- weights first (small, needed immediately), x^T on the
            # gpsimd DMA queue in parallel ----
            xw_ctx = tc.tile_pool(name="xw", bufs=1)
            xw = xw_ctx.__enter__()
            wq = xw.tile([128, NCT, DPC], DT_W)
            wk = xw.tile([128, NCT, DPC], DT_W)
            wv = xw.tile([128, NCT, DPC], DT_W)
            for w_sb, w_h in ((wq, wq_h), (wk, wk_h), (wv, wv_h)):
                nc.sync.dma_start(
                    w_sb[:], w_h[:].rearrange("(ct p) m -> p ct m", p=128))
            xT = xw.tile([128, NCT, B, T], DT_X)
            xsrc = xT_h[:].rearrange("(ct p) b t -> p ct b t", p=128)
            for ct in range(NCT):
                (nc.gpsimd if V3_GDMA else nc.sync).dma_start(xT[:, ct], xsrc[:, ct])

            # ---- persistent activation tiles ----
            qT = ap.tile([DPC, B, T], DT_P)
            kT = ap.tile([DPC, B, T], DT_P)
            vhat = ap.tile([128, B, NST, HPC, 65], DT_P)
            oT = ap.tile([128, NCT, 512], DT_P)
            for b in range(B):
                nc.gpsimd.memset(vhat[:, b, :, :, 64:65], 1.0)

            a2a_in0 = dp.tile([NCORES, 65, 512], DT_A2A, tag="ai0")
            a2a_in1 = dp.tile([NCORES, 65, 512], DT_A2A, tag="ai1")
            a2a_out0 = dp.tile([NCORES, 65, 512], DT_A2A, tag="ao0")
            a2a_out1 = dp.tile([NCORES, 65, 512], DT_A2A, tag="ao1")
            a2a_in = [a2a_in0, a2a_in1]
            a2a_out = [a2a_out0, a2a_out1]

            def proj(b, qb):
                sl = slice(qb * 512, (qb + 1) * 512)
                for w_sb, bias, dst in ((wq, bqT, qT), (wk, bkT, kT)):
                    ps = psM.tile([128, 512], F32, tag="m")
                    for ct in range(NCT):
                        nc.tensor.matmul(ps[:], w_sb[:, ct], xT[:, ct, b, sl],
                                         start=(ct == 0), stop=(ct == NCT - 1))
                    nc.vector.tensor_scalar_add(dst[:, b, sl], ps[:], bias[:])
                # v^T, then transpose 128x128 blocks into s-major vhat
                ps = psM.tile([128, 512], F32, tag="m")
                for ct in range(NCT):
                    nc.tensor.matmul(ps[:], wv[:, ct], xT[:, ct, b, sl],
                                     start=(ct == 0), stop=(ct == NCT - 1))
                vt = vtp.tile([128, 512], DT_P, tag="vt")
                nc.vector.tensor_scalar_add(vt[:], ps[:], bvT[:])
                for sub in range(4):
                    st = qb * 4 + sub
                    tr = psM.tile([128, 128], DT_P, tag="m")
                    nc.tensor.transpose(
                        tr[:], vt[:, sub * 128:(sub + 1) * 128], ident[:])
                    nc.vector.tensor_copy(
                        vhat[:, b, st, :, 0:64],
                        tr[:].rearrange("p (hh d) -> p hh d", hh=HPC))

            def attn(hh, b, qb):
                hlo = hh * 64
                o_ps = psOC.tile([65, 512], F32, tag="o")
                nsi = 4 * qb + 4
                # chunks (si, lo): lo = in-block column offset; pack pairs
                # into one 2-bank PSUM tile so exp covers both
                chunks = [(si, 0) for si in range(4 * qb)] + \
                         [(si, si * 128 - qb * 512) for si in range(4 * qb, nsi)]
                groups = []
                i = 0
                while i < len(chunks):
                    w0 = 512 - chunks[i][1]
                    if V3_PAIR and i + 1 < len(chunks) and w0 + (512 - chunks[i + 1][1]) <= 1024:
                        groups.append([chunks[i], chunks[i + 1]])
                        i += 2
                    else:
                        groups.append([chunks[i]])
                        i += 1
                for grp in groups:
                    tot = sum(512 - lo for _, lo in grp)
                    s_ps = psS2.tile([128, 1024], F32, tag="s2")
                    p_sb = pp.tile([128, 1024], DT_P, tag="p")
                    off = 0
                    for si, lo in grp:
                        w = 512 - lo
                        nc.tensor.matmul(
                            s_ps[:, off:off + w],
                            kT[hlo:hlo + 64, b, si * 128:(si + 1) * 128],
                            qT[hlo:hlo + 64, b, qb * 512 + lo:(qb + 1) * 512],
                            start=True, stop=True)
                        off += w
                    nc.scalar.activation(p_sb[:, 0:tot], s_ps[:, 0:tot],
                                         AF.Exp, scale=0.125)
                    off = 0
                    for si, lo in grp:
                        w = 512 - lo
                        if lo > 0 or si * 128 == qb * 512:
                            # diagonal block: causal triangle mask
                            nc.gpsimd.affine_select(
                                out=p_sb[:, off:off + 128],
                                in_=p_sb[:, off:off + 128],
                                compare_op=OP.is_ge, fill=0.0, base=0,
                                channel_multiplier=-1, pattern=[[1, 128]])
                        nc.tensor.matmul(
                            o_ps[:, lo:512], vhat[:, b, si, hh, :],
                            p_sb[:, off:off + w],
                            start=(si == 0), stop=(si == nsi - 1))
                        off += w
                oh = ohp.tile([65, 512], DT_A2A, tag="oh")
                nc.vector.tensor_copy(oh[:], o_ps[:])
                nc.sync.dma_start(a2a_in[hh][b * 4 + qb, :, :], oh[:])

            def half_prep(j):
                """after AllToAll j: reciprocal of denominators, broadcast,
                and scale this head-half of o^T (rows j*64..j*64+64)."""
                dnm = cp.tile([NCORES, 512], DT_A2A, tag=f"dnm{j}")
                nc.sync.dma_start(dnm[:], a2a_out[j][:, 64, :])
                rl = cp.tile([NCORES, 512], F32, tag=f"rl{j}")
                nc.scalar.activation(rl[:], dnm[:], AF.Ln)
                rde = cp.tile([NCORES, 512], BF16, tag=f"rde{j}")
                nc.scalar.activation(rde[:], rl[:], AF.Exp, scale=-1.0)
                # reshape [8, 512] -> one partition [1, 8, 512] for the K=1 mm
                rd = cp.tile([1, NCORES, 512], BF16, tag=f"rd{j}")
                nc.sync.dma_start(rd[:], rde[:])
                for ft in range(NCT):
                    o_raw = orp.tile([64, 512], DT_A2A, tag="oraw")
                    nc.sync.dma_start(o_raw[:], a2a_out[j][ft, 0:64, :])
                    bch = psM.tile([64, 512], F32, tag="m")
                    nc.tensor.matmul(bch[:], ones1[:, 0:64], rd[0:1, ft, :],
                                     start=True, stop=True)
                    nc.vector.tensor_tensor(
                        oT[j * 64:(j + 1) * 64, ft, :], o_raw[:], bch[:],
                        op=OP.mult)

            # ---- phase 1+2 interleaved; A2A per head ----
            for b in range(B):
                for qb in range(NQB):
                    proj(b, qb)
                    attn(0, b, qb)
            nc.gpsimd.collective_compute(
                "AllToAll", OP.bypass, replica_groups=[list(range(NCORES))],
                ins=[a2a_in[0].opt()], outs=[a2a_out[0].opt()])
            half_prep(0)
            for b in range(B):
                for qb in range(NQB):
                    attn(1, b, qb)
            xw_ctx.__exit__(None, None, None)
            nc.gpsimd.collective_compute(
                "AllToAll", OP.bypass, replica_groups=[list(range(NCORES))],
                ins=[a2a_in[1].opt()], outs=[a2a_out[1].opt()])
            half_prep(1)

            # ---- phase 4: out-proj + bias + LayerNorm on my token slice ----
            lnp_ctx = tc.tile_pool(name="lnp", bufs=2)
            lnp = lnp_ctx.__enter__()
            wo = lnp.tile([128, NCT, C], DT_W, tag="wo")
            nc.gpsimd.dma_start(
                wo[:], wo_h[:].rearrange("(ct p) m -> p ct m", p=128))
            bo = lnp.tile([1, C], BF16, tag="bo")
            nc.sync.dma_start(bo[:], bo_h[:])
            gam = lnp.tile([128, C], BF16, tag="gam")
            nc.sync.dma_start(gam[:], gam_h[:])
            bet = lnp.tile([128, C], BF16, tag="bet")
            nc.sync.dma_start(bet[:], bet_h[:])
            eps_t = lnp.tile([128, 1], F32, tag="eps")
            nc.gpsimd.memset(eps_t[:], EPS)

            for tt in range(TS // 128):
                yc = lnp.tile([128, C], BF16, tag="yc")
                s0 = lnp.tile([128, 1], F32, tag="s0")
                s1 = lnp.tile([128, 1], F32, tag="s1")
                q0 = lnp.tile([128, 1], F32, tag="q0")
                q1 = lnp.tile([128, 1], F32, tag="q1")
                sq = lnp.tile([128, 512], F32, tag="sq")
                for nb, (s_acc, q_acc) in enumerate(((s0, q0), (s1, q1))):
                    y = psM.tile([128, 512], F32, tag="m")
                    for ft in range(NCT):
                        nc.tensor.matmul(
                            y[:], oT[:, ft, tt * 128:(tt + 1) * 128],
                            wo[:, ft, nb * 512:(nb + 1) * 512],
                            start=(ft == 0), stop=False)
                    nc.tensor.matmul(y[:], ones1[:], bo[:, nb * 512:(nb + 1) * 512],
                                     start=False, stop=True)
                    half = slice(nb * 512, (nb + 1) * 512)
                    if V3_LN:
                        # move to SBUF + row-sum on the scalar engine
                        nc.scalar.activation(yc[:, half], y[:], AF.Identity,
                                             accum_out=s_acc[:])
                        # sum of squares on the scalar engine
                        nc.scalar.activation(sq[:], y[:], AF.Square,
                                             accum_out=q_acc[:])
                    else:
                        nc.vector.tensor_reduce(
                            s_acc[:], y[:], axis=mybir.AxisListType.X, op=OP.add)
                        nc.scalar.activation(sq[:], y[:], AF.Square,
                                             accum_out=q_acc[:])
                        nc.vector.tensor_copy(yc[:, half], y[:])
                mu = lnp.tile([128, 1], F32, tag="mu")
                nc.vector.tensor_tensor(mu[:], s0[:], s1[:], op=OP.add)
                nc.vector.tensor_scalar_mul(mu[:], mu[:], 1.0 / C)
                var = lnp.tile([128, 1], F32, tag="var")
                nc.vector.tensor_tensor(var[:], q0[:], q1[:], op=OP.add)
                nc.vector.tensor_scalar_mul(var[:], var[:], 1.0 / C)
                m2 = lnp.tile([128, 1], F32, tag="m2")
                nc.vector.tensor_tensor(m2[:], mu[:], mu[:], op=OP.mult)
                nc.vector.tensor_tensor(var[:], var[:], m2[:], op=OP.subtract)
                sd = lnp.tile([128, 1], F32, tag="sd")
                nc.scalar.activation(sd[:], var[:], AF.Sqrt, bias=eps_t[:])
                istd = lnp.tile([128, 1], F32, tag="istd")
                nc.vector.reciprocal(istd[:], sd[:])
                yn = lnp.tile([128, C], BF16, tag="yn")
                nc.vector.tensor_scalar(
                    yn[:], yc[:], mu[:], istd[:], op0=OP.subtract, op1=OP.mult)
                yg = lnp.tile([128, C], BF16, tag="yg")
                nc.vector.tensor_tensor(yg[:], yn[:], gam[:], op=OP.mult)
                yf = lnp.tile([128, C], BF16, tag="yf")
                nc.vector.tensor_tensor(yf[:], yg[:], bet[:], op=OP.add)
                nc.sync.dma_start(out_h[tt * 128:(tt + 1) * 128, :], yf[:])
            lnp_ctx.__exit__(None, None, None)

    nc.compile()
    return nc


def _get_nc():
    if "nc" not in _CACHE:
        _CACHE["nc"] = _build()
    return _CACHE["nc"]


def _make_in_maps(inputs):
    x = np.asarray(inputs["x"], np.float32)
    Wq = np.asarray(inputs["Wq"], np.float32)
    Wk = np.asarray(inputs["Wk"], np.float32)
    Wv = np.asarray(inputs["Wv"], np.float32)
    Wo = np.asarray(inputs["Wo"], np.float32)
    bq = np.asarray(inputs["bq"], np.float32)
    bk = np.asarray(inputs["bk"], np.float32)
    bv = np.asarray(inputs["bv"], np.float32)
    bo = np.asarray(inputs["bo"], np.float32)
    gamma = np.asarray(inputs["gamma"], np.float32)
    beta = np.asarray(inputs["beta"], np.float32)

    xT = np.ascontiguousarray(x.transpose(2, 0, 1)).astype(NP_X)  # [C, B, T]
    wo_c = np.ascontiguousarray(Wo).astype(NP_W)
    bo_row = np.ascontiguousarray(bo.reshape(1, C)).astype(ml_dtypes.bfloat16)
    gamb = np.ascontiguousarray(np.broadcast_to(gamma, (128, C))).astype(ml_dtypes.bfloat16)
    betb = np.ascontiguousarray(np.broadcast_to(beta, (128, C))).astype(ml_dtypes.bfloat16)

    maps = []
    for i in range(NCORES):
        cols = slice(DPC * i, DPC * (i + 1))
        maps.append({
            "xT": xT,
            "wq": np.ascontiguousarray(Wq[:, cols]).astype(NP_W),
            "wk": np.ascontiguousarray(Wk[:, cols]).astype(NP_W),
            "wv": np.ascontiguousarray(Wv[:, cols]).astype(NP_W),
            "wo": wo_c,
            "bqT": np.ascontiguousarray(bq[cols].reshape(DPC, 1)),
            "bkT": np.ascontiguousarray(bk[cols].reshape(DPC, 1)),
            "bvT": np.ascontiguousarray(bv[cols].reshape(DPC, 1)),
            "bo_row": bo_row,
            "gamb": gamb,
            "betb": betb,
        })
    return maps


def _run(inputs, trace=False, **kwargs):
    nc = _get_nc()
    in_maps = _make_in_maps(inputs)
    res = run_bass_kernel_spmd(nc, in_maps, core_ids=list(range(NCORES)),
                               trace=trace, **kwargs)
    y = np.empty((B, T, C), np.float32)
    for i in range(NCORES):
        b, ts = divmod(i, 4)
        y[b, ts * TS:(ts + 1) * TS, :] = res.results[i]["out"].astype(np.float32)
    return y, res


def kernel(**inputs) -> np.ndarray:
    out, _ = _run(inputs, trace=False)
    return out
